# revision 5
# baseline (speedup 1.0000x reference)
"""MemoryAttention Trainium2 Bass kernel.

Problem (hardcoded shapes): b=4, nq=nk=2048, d_model=512, n_heads=8,
d_k=64, n_mem=64 memory slots appended to keys/values.

Sharding: 8 cores = (batch i in 0..3) x (head-group g in 0..1, 4 heads each).
Each core computes, for its batch and its 4 heads:
  qh = q @ Wq_g.T + bq_g           (stored transposed: QT [dims, nq])
  kc = [k @ Wk_g.T + bk_g ; m_k*8] (stored transposed: KT [dims, 2176])
  vc = [v @ Wv_g.T + bv_g ; m_v*8] (natural: VC [keys, dims] + ones col)
  S^T[k,q] = KT_h.T @ QT_h ; p~ = exp(S^T * scale_k + bias_k)  (mask/weights
             folded into per-key scale/bias of the Exp activation)
  pv[d,q]  = VC_h'.T @ p~  (extra ones-column gives the softmax denominator)
  nout     = pv[0:64] / denom
  partial_out[q,:] += nout_h.T @ WoT_h    (accumulated over the 4 heads)
Host sums the two head-group partials per batch and adds bo.

All matmuls run in float32r (fp32 with 11-bit-mantissa RNE rounding on
ingest, fp32 accumulate) - 1 cycle/row on the PE for moving dim >= 256.

Self-contained: no file reads, shapes hardcoded.
"""

import numpy as np

import concourse.bass as bass
import concourse.tile as tile
import concourse.mybir as mybir
from concourse import bacc
from concourse.bass_utils import run_bass_kernel_spmd

F32 = mybir.dt.float32
F32R = mybir.dt.float32r
AF = mybir.ActivationFunctionType

D_MODEL = 512
N_HEADS = 8
N_MEM = 64
DK = 64
B = 4
NQ = 2048
NK = 2048
NKP = 2176           # keys padded: 2048 real + 64 memory + 64 zero-pad
KT_TILES = NKP // 128  # 17
NEG = -1.0e30

N_CORES = 8
HPG = 4              # heads per group
GD = HPG * DK        # 256 dims per group


def build_nc():
    nc = bacc.Bacc("TRN2", target_bir_lowering=False, debug=False)

    qT = nc.dram_tensor("qT", [D_MODEL, NQ], F32R, kind="ExternalInput").ap()
    kT = nc.dram_tensor("kT", [D_MODEL, NK], F32R, kind="ExternalInput").ap()
    vT = nc.dram_tensor("vT", [D_MODEL, NK], F32R, kind="ExternalInput").ap()
    wqT = nc.dram_tensor("wqT", [D_MODEL, GD], F32R, kind="ExternalInput").ap()
    wkT = nc.dram_tensor("wkT", [D_MODEL, GD], F32R, kind="ExternalInput").ap()
    wvT = nc.dram_tensor("wvT", [D_MODEL, GD], F32R, kind="ExternalInput").ap()
    woT = nc.dram_tensor("woT", [GD, D_MODEL], F32R, kind="ExternalInput").ap()
    bq2 = nc.dram_tensor("bq2", [128, 2], F32, kind="ExternalInput").ap()
    bk2 = nc.dram_tensor("bk2", [128, 2], F32, kind="ExternalInput").ap()
    bvb = nc.dram_tensor("bvb", [GD], F32, kind="ExternalInput").ap()
    mkT = nc.dram_tensor("mkT", [2, 128, 128], F32R, kind="ExternalInput").ap()
    mv16 = nc.dram_tensor("mv16", [128, HPG, 65], F32R, kind="ExternalInput").ap()
    onesd = nc.dram_tensor("onesd", [1], F32R, kind="ExternalInput").ap()
    scalev = nc.dram_tensor("scalev", [NKP], F32, kind="ExternalInput").ap()
    biasv = nc.dram_tensor("biasv", [NKP], F32, kind="ExternalInput").ap()
    out = nc.dram_tensor("out", [NQ, D_MODEL], F32, kind="ExternalOutput").ap()

    with tile.TileContext(nc) as tc:
        with tc.tile_pool(name="const", bufs=1) as const, \
             tc.tile_pool(name="stage", bufs=1) as stage, \
             tc.tile_pool(name="expp", bufs=3) as expp, \
             tc.tile_pool(name="noutp", bufs=2) as noutp, \
             tc.tile_pool(name="recp", bufs=2) as recp, \
             tc.tile_pool(name="outp", bufs=3) as outp, \
             tc.tile_pool(name="dramp", bufs=4, space="DRAM") as dramp, \
             tc.tile_pool(name="ps_st", bufs=3, space="PSUM") as ps_st, \
             tc.tile_pool(name="ps_pv", bufs=2, space="PSUM") as ps_pv, \
             tc.tile_pool(name="ps_big", bufs=2, space="PSUM") as ps_big:

            # ---- load constants / weights ----
            wq_sb = const.tile([128, 4, GD], F32R, tag="wq")
            wk_sb = const.tile([128, 4, GD], F32R, tag="wk")
            wv_sb = const.tile([128, 4, GD], F32R, tag="wv")
            wo_sb = const.tile([64, HPG, D_MODEL], F32R, tag="wo")
            nc.sync.dma_start(wq_sb[:], wqT.rearrange("(ic p) m -> p ic m", p=128))
            nc.sync.dma_start(wk_sb[:], wkT.rearrange("(ic p) m -> p ic m", p=128))
            nc.sync.dma_start(wv_sb[:], wvT.rearrange("(ic p) m -> p ic m", p=128))
            nc.sync.dma_start(wo_sb[:], woT.rearrange("(h p) n -> p h n", p=64))
            bq_sb = const.tile([128, 2], F32, tag="bq")
            bk_sb = const.tile([128, 2], F32, tag="bk")
            nc.sync.dma_start(bq_sb[:], bq2)
            nc.sync.dma_start(bk_sb[:], bk2)
            bvb_sb = const.tile([128, HPG, DK], F32, tag="bvb")
            nc.sync.dma_start(
                bvb_sb[:],
                bvb.rearrange("(h d) -> h d", h=HPG).unsqueeze(0)
                   .broadcast_to([128, HPG, DK]))
            scale_sb = const.tile([128, KT_TILES], F32, tag="scale")
            bias_sb = const.tile([128, KT_TILES], F32, tag="bias")
            nc.sync.dma_start(scale_sb[:], scalev.rearrange("(t p) -> p t", p=128))
            nc.sync.dma_start(bias_sb[:], biasv.rearrange("(t p) -> p t", p=128))

            # ---- persistent projected tensors ----
            QT = [const.tile([128, NQ], F32R, tag=f"QT{c}", name=f"QT{c}")
                  for c in range(2)]
            KT = [const.tile([128, NKP], F32R, tag=f"KT{c}", name=f"KT{c}")
                  for c in range(2)]
            VC = const.tile([128, KT_TILES, HPG, 65], F32R, tag="VC")

            for c in range(2):
                nc.sync.dma_start(KT[c][:, NK:NKP], mkT[c])
            # ones columns for the denominator (tiles 0-15); tile 16 comes
            # fully from the host (memory rows + ones col + zero pad rows)
            for h in range(HPG):
                nc.sync.dma_start(
                    VC[:, 0:16, h, 64:65],
                    onesd.unsqueeze(0).unsqueeze(0)
                         .broadcast_to([128, 16, 1]))
            nc.sync.dma_start(VC[:, 16, :, :], mv16)

            # ---- stage q/k/v (transposed) ----
            qs = [stage.tile([128, NQ], F32R, tag=f"qs{ic}", name=f"qs{ic}")
                  for ic in range(4)]
            ks = [stage.tile([128, NK], F32R, tag=f"ks{ic}", name=f"ks{ic}")
                  for ic in range(4)]
            vs = [stage.tile([128, NK], F32R, tag=f"vs{ic}", name=f"vs{ic}")
                  for ic in range(4)]
            for ic in range(4):
                nc.sync.dma_start(qs[ic][:], qT[bass.ts(ic, 128), :])
                nc.sync.dma_start(ks[ic][:], kT[bass.ts(ic, 128), :])
                nc.sync.dma_start(vs[ic][:], vT[bass.ts(ic, 128), :])

            # ---- projections ----
            for c in range(2):
                for j in range(NQ // 512):
                    ps = ps_big.tile([128, 512], F32, tag="big")
                    for ic in range(4):
                        nc.tensor.matmul(ps[:], wq_sb[:, ic, bass.ts(c, 128)],
                                         qs[ic][:, bass.ts(j, 512)],
                                         start=(ic == 0), stop=(ic == 3))
                    nc.scalar.add(QT[c][:, bass.ts(j, 512)], ps[:],
                                  bq_sb[:, c:c + 1])
                for j in range(NK // 512):
                    ps = ps_big.tile([128, 512], F32, tag="big")
                    for ic in range(4):
                        nc.tensor.matmul(ps[:], wk_sb[:, ic, bass.ts(c, 128)],
                                         ks[ic][:, bass.ts(j, 512)],
                                         start=(ic == 0), stop=(ic == 3))
                    nc.scalar.add(KT[c][:, bass.ts(j, 512)], ps[:],
                                  bk_sb[:, c:c + 1])
            for kt in range(NK // 128):
                ps = ps_big.tile([128, GD], F32, tag="big")
                for ic in range(4):
                    nc.tensor.matmul(ps[:], vs[ic][:, bass.ts(kt, 128)],
                                     wv_sb[:, ic, :],
                                     start=(ic == 0), stop=(ic == 3))
                nc.vector.tensor_add(VC[:, kt, :, 0:64],
                                     ps[:].rearrange("p (h d) -> p h d", h=HPG),
                                     bvb_sb[:])

            # ---- attention ----
            for j in range(NQ // 512):
                nouts = []
                for h in range(HPG):
                    c, r = divmod(h, 2)
                    base = 64 * r
                    pv = ps_pv.tile([65, 512], F32, tag="pv")
                    for kt in range(KT_TILES):
                        st = ps_st.tile([128, 512], F32, tag="st")
                        nc.tensor.matmul(
                            st[:],
                            KT[c][base:base + 64, bass.ts(kt, 128)],
                            QT[c][base:base + 64, bass.ts(j, 512)],
                            start=True, stop=True)
                        ex = expp.tile([128, 512], F32R, tag="ex")
                        nc.scalar.activation(
                            ex[:], st[:], AF.Exp,
                            bias=bias_sb[:, kt:kt + 1],
                            scale=scale_sb[:, kt:kt + 1])
                        nc.tensor.matmul(pv[:], VC[:, kt, h, :], ex[:],
                                         start=(kt == 0),
                                         stop=(kt == KT_TILES - 1))
                    rec = recp.tile([65, 512], F32, tag="rec")
                    nc.vector.reciprocal(rec[64:65, :], pv[64:65, :])
                    recd = dramp.tile([1, 512], F32, tag="recd")
                    nc.sync.dma_start(recd[:], rec[64:65, :])
                    rb = recp.tile([64, 512], F32, tag="rb")
                    nc.sync.dma_start(rb[:], recd[:].broadcast_to([64, 512]))
                    no = noutp.tile([64, 512], F32R, tag=f"no{h}")
                    nc.vector.tensor_mul(no[:], pv[0:64, :], rb[:])
                    nouts.append(no)
                for qc in range(4):
                    pf = ps_big.tile([128, 512], F32, tag="big")
                    for h in range(HPG):
                        nc.tensor.matmul(pf[:],
                                         nouts[h][:, bass.ts(qc, 128)],
                                         wo_sb[:, h, :],
                                         start=(h == 0), stop=(h == HPG - 1))
                    ob = outp.tile([128, 512], F32, tag="ob")
                    nc.vector.tensor_copy(ob[:], pf[:])
                    nc.sync.dma_start(out[j * 512 + qc * 128:
                                          j * 512 + (qc + 1) * 128, :], ob[:])

    nc.compile()
    return nc


_NC = None


def get_nc():
    global _NC
    if _NC is None:
        _NC = build_nc()
    return _NC


def make_in_maps(q, k, v, attention_mask, attention_weights,
                 Wq, bq, Wk, bk, Wv, bv, Wo, bo, m_k, m_v):
    q = np.asarray(q, np.float32)
    k = np.asarray(k, np.float32)
    v = np.asarray(v, np.float32)
    Wq = np.asarray(Wq, np.float32)
    Wk = np.asarray(Wk, np.float32)
    Wv = np.asarray(Wv, np.float32)
    Wo = np.asarray(Wo, np.float32)
    m_k = np.asarray(m_k, np.float32)
    m_v = np.asarray(m_v, np.float32)
    aw = np.asarray(attention_weights, np.float32).reshape(B, NK)
    am = np.asarray(attention_mask).reshape(B, NK)

    mks = m_k[0] * np.float32(np.sqrt(DK))      # [64, 512]
    mvs = m_v[0] * np.float32(np.sqrt(N_MEM))   # [64, 512]

    in_maps = []
    qTs = [np.ascontiguousarray(q[i].T) for i in range(B)]
    kTs = [np.ascontiguousarray(k[i].T) for i in range(B)]
    vTs = [np.ascontiguousarray(v[i].T) for i in range(B)]
    for i in range(B):
        scalev = np.empty(NKP, np.float32)
        biasv = np.empty(NKP, np.float32)
        scalev[:NK] = aw[i] * np.float32(1.0 / 8.0)
        scalev[NK:NK + N_MEM] = np.float32(1.0 / 8.0)
        scalev[NK + N_MEM:] = 0.0
        biasv[:NK] = np.where(am[i] != 0, np.float32(NEG), np.float32(0.0))
        biasv[NK:NK + N_MEM] = 0.0
        biasv[NK + N_MEM:] = np.float32(NEG)
        for g in range(2):
            sl = slice(g * GD, (g + 1) * GD)
            mkT_full = np.zeros((2, 128, 128), np.float32)
            mkT_full[:, :, :N_MEM] = mks[:, sl].T.reshape(2, 128, N_MEM)
            mv16_full = np.zeros((128, HPG, 65), np.float32)
            mv16_full[:N_MEM, :, :DK] = mvs[:, sl].reshape(N_MEM, HPG, DK)
            mv16_full[:N_MEM, :, DK] = 1.0
            in_maps.append(dict(
                qT=qTs[i], kT=kTs[i], vT=vTs[i],
                wqT=np.ascontiguousarray(Wq[sl, :].T),
                wkT=np.ascontiguousarray(Wk[sl, :].T),
                wvT=np.ascontiguousarray(Wv[sl, :].T),
                woT=np.ascontiguousarray(Wo[:, sl].T),
                bq2=np.ascontiguousarray(
                    np.asarray(bq, np.float32)[sl].reshape(2, 128).T),
                bk2=np.ascontiguousarray(
                    np.asarray(bk, np.float32)[sl].reshape(2, 128).T),
                bvb=np.ascontiguousarray(np.asarray(bv, np.float32)[sl]),
                mkT=mkT_full,
                mv16=mv16_full,
                onesd=np.ones(1, np.float32),
                scalev=scalev, biasv=biasv,
            ))
    return in_maps


def assemble(results, bo):
    bo = np.asarray(bo, np.float32)
    out = np.empty((B, NQ, D_MODEL), np.float32)
    for i in range(B):
        out[i] = results[2 * i]["out"] + results[2 * i + 1]["out"] + bo
    return out


def kernel(**inputs):
    nc = get_nc()
    in_maps = make_in_maps(**{k2: v2 for k2, v2 in inputs.items()})
    res = run_bass_kernel_spmd(nc, in_maps, core_ids=list(range(N_CORES)))
    return assemble(res.results, inputs["bo"])


# revision 13
# speedup vs baseline: 1.0159x; 1.0159x over previous
"""MemoryAttention Trainium2 Bass kernel.

Problem (hardcoded): b=4, nq=nk=2048, d_model=512, n_heads=8, d_k=64,
n_mem=64 memory slots appended to keys/values, per-key attention weights,
and a key mask (mask==1 -> -inf before softmax).

Sharding: 8 cores = (batch i in 0..3) x (head-group g in 0..1, 4 heads).
Host sums the two head-group partials per batch and adds bo.

Key trick: masked keys contribute exactly zero to the softmax, and the mask
is a kernel input - so the host COMPACTS keys per batch (keeps only the
~50% unmasked keys), cutting attention work ~2x. The kernel is compiled for
a fixed padded key count; a full-width variant is compiled as fallback if a
batch ever has too many unmasked keys.

Per-core pipeline (all matmuls float32r: 11-bit-mantissa RNE on ingest,
fp32 accumulate, 1 PE cycle/row):
  QT[dims,nq]   = Wq_g @ q^T + bq   (x^T supplied by host)
  KT[dims,kc]   = [Wk_g @ k_kept^T + bk | m_k*8 | 0]
  VC[keys,dims] = [v_kept @ Wv_g^T + bv ; m_v*8 ; 0] plus a ones column
  per (head, 1024-query tile, 128-key tile):
    S^T = KT_h.T @ QT_h                (PSUM, 2 matmuls of N=512)
    p~  = Exp(S^T * scale_k + bias_k)  (ONE 1024-wide ACT op; per-key
          attention weight & 1/sqrt(dk) in scale, mask/pad -1e30 in bias)
    pv += VC_h'.T @ p~                 (ones column accumulates denominator)
  nout = pv[0:64] * recip(denom)       (DVE; recip broadcast via GpSimd)
  out_partial += nout_pair.T @ WoT     (head pairs packed to K=128)

Self-contained: no file reads, shapes hardcoded.
"""

import numpy as np

import concourse.bass as bass
import concourse.tile as tile
import concourse.mybir as mybir
from concourse import bacc
from concourse.bass_utils import run_bass_kernel_spmd

F32 = mybir.dt.float32
F32R = mybir.dt.float32r
AF = mybir.ActivationFunctionType
ts = bass.ts

D_MODEL = 512
N_HEADS = 8
N_MEM = 64
DK = 64
B = 4
NQ = 2048
NK = 2048
NEG = -1.0e30

N_CORES = 8
HPG = 4              # heads per group
GD = HPG * DK        # 256 dims per group

KPROJ_COMPACT = 1152   # unmasked-key capacity (mean 1024, +5.7 sigma)


def build_nc(kproj):
    """kproj = projected-key columns (multiple of 128). Key layout:
    [0:kproj) compacted keys (+zero pad), [kproj:kproj+64) memory slots,
    [kproj+64:kproj+128) zero pad."""
    nkp = kproj + 128
    nkt = nkp // 128

    nc = bacc.Bacc("TRN2", target_bir_lowering=False, debug=False)

    qT = nc.dram_tensor("qT", [D_MODEL, NQ], F32R, kind="ExternalInput").ap()
    kT = nc.dram_tensor("kT", [D_MODEL, kproj], F32R, kind="ExternalInput").ap()
    vT = nc.dram_tensor("vT", [D_MODEL, kproj], F32R, kind="ExternalInput").ap()
    wqT = nc.dram_tensor("wqT", [D_MODEL, GD], F32R, kind="ExternalInput").ap()
    wkT = nc.dram_tensor("wkT", [D_MODEL, GD], F32R, kind="ExternalInput").ap()
    wvT = nc.dram_tensor("wvT", [D_MODEL, GD], F32R, kind="ExternalInput").ap()
    woT = nc.dram_tensor("woT", [GD, D_MODEL], F32R, kind="ExternalInput").ap()
    bq2 = nc.dram_tensor("bq2", [128, 2], F32, kind="ExternalInput").ap()
    bk2 = nc.dram_tensor("bk2", [128, 2], F32, kind="ExternalInput").ap()
    bvb = nc.dram_tensor("bvb", [GD], F32, kind="ExternalInput").ap()
    mkT = nc.dram_tensor("mkT", [2, 128, 128], F32R, kind="ExternalInput").ap()
    mv16 = nc.dram_tensor("mv16", [128, HPG, 65], F32R, kind="ExternalInput").ap()
    onesd = nc.dram_tensor("onesd", [1], F32R, kind="ExternalInput").ap()
    scalev = nc.dram_tensor("scalev", [nkp], F32, kind="ExternalInput").ap()
    biasv = nc.dram_tensor("biasv", [nkp], F32, kind="ExternalInput").ap()
    out = nc.dram_tensor("out", [NQ, D_MODEL], F32, kind="ExternalOutput").ap()

    with tile.TileContext(nc) as tc:
        with tc.tile_pool(name="const", bufs=1) as const, \
             tc.tile_pool(name="stage", bufs=1) as stage, \
             tc.tile_pool(name="expp", bufs=4) as expp, \
             tc.tile_pool(name="noutp", bufs=2) as noutp, \
             tc.tile_pool(name="recp", bufs=2) as recp, \
             tc.tile_pool(name="outp", bufs=3) as outp, \
             tc.tile_pool(name="ps_st", bufs=2, space="PSUM") as ps_st, \
             tc.tile_pool(name="ps_pv", bufs=2, space="PSUM") as ps_pv:

            # ---- weights / constants ----
            wq_sb = const.tile([128, 4, GD], F32R, tag="wq")
            wk_sb = const.tile([128, 4, GD], F32R, tag="wk")
            wv_sb = const.tile([128, 4, GD], F32R, tag="wv")
            wo_sb = const.tile([128, 2, D_MODEL], F32R, tag="wo")
            nc.sync.dma_start(wq_sb[:], wqT.rearrange("(ic p) m -> p ic m", p=128))
            nc.sync.dma_start(wk_sb[:], wkT.rearrange("(ic p) m -> p ic m", p=128))
            nc.sync.dma_start(wv_sb[:], wvT.rearrange("(ic p) m -> p ic m", p=128))
            nc.sync.dma_start(wo_sb[:], woT.rearrange("(c p) n -> p c n", p=128))
            bq_sb = const.tile([128, 2], F32, tag="bq")
            bk_sb = const.tile([128, 2], F32, tag="bk")
            nc.sync.dma_start(bq_sb[:], bq2)
            nc.sync.dma_start(bk_sb[:], bk2)
            bvb_sb = const.tile([128, HPG, DK], F32, tag="bvb")
            nc.sync.dma_start(
                bvb_sb[:],
                bvb.rearrange("(h d) -> h d", h=HPG).unsqueeze(0)
                   .broadcast_to([128, HPG, DK]))
            scale_sb = const.tile([128, nkt], F32, tag="scale")
            bias_sb = const.tile([128, nkt], F32, tag="bias")
            nc.sync.dma_start(scale_sb[:], scalev.rearrange("(t p) -> p t", p=128))
            nc.sync.dma_start(bias_sb[:], biasv.rearrange("(t p) -> p t", p=128))

            # ---- persistent projected tensors ----
            QT = [const.tile([128, NQ], F32R, tag=f"QT{c}", name=f"QT{c}")
                  for c in range(2)]
            KT = [const.tile([128, nkp], F32R, tag=f"KT{c}", name=f"KT{c}")
                  for c in range(2)]
            VC = const.tile([128, nkt, HPG, 65], F32R, tag="VC")

            for c in range(2):
                nc.sync.dma_start(KT[c][:, kproj:nkp], mkT[c])
            for h in range(HPG):
                nc.sync.dma_start(
                    VC[:, 0:nkt - 1, h, 64:65],
                    onesd.unsqueeze(0).unsqueeze(0)
                         .broadcast_to([128, nkt - 1, 1]))
            nc.sync.dma_start(VC[:, nkt - 1, :, :], mv16)

            # ---- stage q/k/v (host-transposed) ----
            qs = [stage.tile([128, NQ], F32R, tag=f"qs{ic}", name=f"qs{ic}")
                  for ic in range(4)]
            ks = [stage.tile([128, kproj], F32R, tag=f"ks{ic}", name=f"ks{ic}")
                  for ic in range(4)]
            vs = [stage.tile([128, kproj], F32R, tag=f"vs{ic}", name=f"vs{ic}")
                  for ic in range(4)]
            for ic in range(4):
                nc.sync.dma_start(ks[ic][:], kT[ts(ic, 128), :])
            for ic in range(4):
                nc.sync.dma_start(vs[ic][:], vT[ts(ic, 128), :])
            for ic in range(4):
                nc.sync.dma_start(qs[ic][:], qT[ts(ic, 128), :])

            # ---- projections ----
            # KT/VC first: they gate the whole attention phase; QT last
            # (q is the largest input DMA and only gates its own J-tile).
            for c in range(2):
                j0 = 0
                while j0 < kproj:
                    jb = min(512, kproj - j0)
                    ps = ps_pv.tile([128, jb], F32, tag="pv", name="psk")
                    for ic in range(4):
                        nc.tensor.matmul(ps[:], wk_sb[:, ic, ts(c, 128)],
                                         ks[ic][:, j0:j0 + jb],
                                         start=(ic == 0), stop=(ic == 3))
                    nc.vector.tensor_scalar_add(KT[c][:, j0:j0 + jb], ps[:],
                                                bk_sb[:, c:c + 1])
                    j0 += jb
            for kt in range(kproj // 128):
                ps = ps_pv.tile([128, GD], F32, tag="pv", name="psv")
                for ic in range(4):
                    nc.tensor.matmul(ps[:], vs[ic][:, ts(kt, 128)],
                                     wv_sb[:, ic, :],
                                     start=(ic == 0), stop=(ic == 3))
                nc.vector.tensor_add(VC[:, kt, :, 0:64],
                                     ps[:].rearrange("p (h d) -> p h d", h=HPG),
                                     bvb_sb[:])
            for c in range(2):
                for j in range(NQ // 512):
                    ps = ps_pv.tile([128, 512], F32, tag="pv", name="psq")
                    for ic in range(4):
                        nc.tensor.matmul(ps[:], wq_sb[:, ic, ts(c, 128)],
                                         qs[ic][:, ts(j, 512)],
                                         start=(ic == 0), stop=(ic == 3))
                    nc.vector.tensor_scalar_add(QT[c][:, ts(j, 512)], ps[:],
                                                bq_sb[:, c:c + 1])
            # ---- attention ----
            for J in range(NQ // 1024):
                q0 = J * 1024
                nops = [noutp.tile([128, 1024], F32R, tag=f"nop{c}",
                                   name=f"nop{c}") for c in range(2)]
                for h in (1, 3, 0, 2):
                    c, r = divmod(h, 2)
                    base = 64 * r
                    pv = ps_pv.tile([65, 1024], F32, tag="pv", name="pv")
                    for kt in range(nkt):
                        st = ps_st.tile([128, 1024], F32, tag="st", name="st")
                        for u in range(2):
                            nc.tensor.matmul(
                                st[:, ts(u, 512)],
                                KT[c][base:base + 64, ts(kt, 128)],
                                QT[c][base:base + 64,
                                      q0 + u * 512:q0 + (u + 1) * 512],
                                start=True, stop=True)
                        ex = expp.tile([128, 1024], F32R, tag="ex", name="ex")
                        nc.scalar.activation(
                            ex[:], st[:], AF.Exp,
                            bias=bias_sb[:, kt:kt + 1],
                            scale=scale_sb[:, kt:kt + 1])
                        for u in range(2):
                            nc.tensor.matmul(pv[:, ts(u, 512)],
                                             VC[:, kt, h, :], ex[:, ts(u, 512)],
                                             start=(kt == 0),
                                             stop=(kt == nkt - 1))
                    rec = recp.tile([65, 1024], F32, tag="rec", name="rec")
                    nc.vector.reciprocal(rec[64:65, :], pv[64:65, :])
                    # partition_broadcast ucode only reads partition 0 on HW:
                    # hop the reciprocal row down to partition 0 via DMA first
                    rec0 = recp.tile([1, 1024], F32, tag="rec0", name="rec0")
                    nc.gpsimd.dma_start(rec0[:], rec[64:65, :])
                    rb = recp.tile([64, 1024], F32, tag="rb", name="rb")
                    nc.gpsimd.partition_broadcast(rb[:], rec0[0:1, :])
                    if r == 0:
                        nc.vector.tensor_mul(nops[c][0:64, :], pv[0:64, :],
                                             rb[:])
                    else:
                        sc = recp.tile([64, 1024], F32R, tag="sc", name="sc")
                        nc.vector.tensor_mul(sc[:], pv[0:64, :], rb[:])
                        nc.gpsimd.dma_start(nops[c][64:128, :], sc[:])
                for qc in range(8):
                    pf = ps_pv.tile([128, 512], F32, tag="pv", name="pf")
                    for c in range(2):
                        nc.tensor.matmul(pf[:], nops[c][:, ts(qc, 128)],
                                         wo_sb[:, c, :],
                                         start=(c == 0), stop=(c == 1))
                    ob = outp.tile([128, 512], F32, tag="ob", name="ob")
                    nc.scalar.copy(ob[:], pf[:])
                    nc.sync.dma_start(out[q0 + qc * 128:q0 + (qc + 1) * 128, :],
                                      ob[:])

    nc.compile()
    return nc


_NCS = {}


def get_nc(kproj=KPROJ_COMPACT):
    if kproj not in _NCS:
        _NCS[kproj] = build_nc(kproj)
    return _NCS[kproj]


def make_in_maps(kproj, q, k, v, attention_mask, attention_weights,
                 Wq, bq, Wk, bk, Wv, bv, Wo, bo, m_k, m_v):
    q = np.asarray(q, np.float32)
    k = np.asarray(k, np.float32)
    v = np.asarray(v, np.float32)
    Wq, Wk, Wv, Wo = (np.asarray(x, np.float32) for x in (Wq, Wk, Wv, Wo))
    m_k = np.asarray(m_k, np.float32)
    m_v = np.asarray(m_v, np.float32)
    aw = np.asarray(attention_weights, np.float32).reshape(B, NK)
    am = np.asarray(attention_mask).reshape(B, NK)
    nkp = kproj + 128
    compact = kproj < NK

    mks = m_k[0] * np.float32(np.sqrt(DK))      # [64, 512]
    mvs = m_v[0] * np.float32(np.sqrt(N_MEM))   # [64, 512]

    in_maps = []
    for i in range(B):
        if compact:
            idx = np.flatnonzero(am[i] == 0)
        else:
            idx = np.arange(NK)
        nkeep = len(idx)
        assert nkeep <= kproj, (nkeep, kproj)
        kTc = np.zeros((D_MODEL, kproj), np.float32)
        vTc = np.zeros((D_MODEL, kproj), np.float32)
        kTc[:, :nkeep] = k[i].T[:, idx]
        vTc[:, :nkeep] = v[i].T[:, idx]
        qTc = np.ascontiguousarray(q[i].T)

        scalev = np.zeros(nkp, np.float32)
        biasv = np.full(nkp, np.float32(NEG), np.float32)
        scalev[:nkeep] = aw[i, idx] * np.float32(1.0 / 8.0)
        biasv[:nkeep] = np.where(am[i, idx] != 0, np.float32(NEG), 0.0)
        scalev[kproj:kproj + N_MEM] = np.float32(1.0 / 8.0)
        biasv[kproj:kproj + N_MEM] = 0.0

        for g in range(2):
            sl = slice(g * GD, (g + 1) * GD)
            mkT_full = np.zeros((2, 128, 128), np.float32)
            mkT_full[:, :, :N_MEM] = mks[:, sl].T.reshape(2, 128, N_MEM)
            mv16_full = np.zeros((128, HPG, 65), np.float32)
            mv16_full[:N_MEM, :, :DK] = mvs[:, sl].reshape(N_MEM, HPG, DK)
            mv16_full[:N_MEM, :, DK] = 1.0
            in_maps.append(dict(
                qT=qTc, kT=kTc, vT=vTc,
                wqT=np.ascontiguousarray(Wq[sl, :].T),
                wkT=np.ascontiguousarray(Wk[sl, :].T),
                wvT=np.ascontiguousarray(Wv[sl, :].T),
                woT=np.ascontiguousarray(Wo[:, sl].T),
                bq2=np.ascontiguousarray(
                    np.asarray(bq, np.float32)[sl].reshape(2, 128).T),
                bk2=np.ascontiguousarray(
                    np.asarray(bk, np.float32)[sl].reshape(2, 128).T),
                bvb=np.ascontiguousarray(np.asarray(bv, np.float32)[sl]),
                mkT=mkT_full,
                mv16=mv16_full,
                onesd=np.ones(1, np.float32),
                scalev=scalev, biasv=biasv,
            ))
    return in_maps


def pick_kproj(attention_mask):
    am = np.asarray(attention_mask).reshape(B, NK)
    max_keep = max(int((am[i] == 0).sum()) for i in range(B))
    return KPROJ_COMPACT if max_keep <= KPROJ_COMPACT else NK


def assemble(results, bo):
    bo = np.asarray(bo, np.float32)
    out = np.empty((B, NQ, D_MODEL), np.float32)
    for i in range(B):
        out[i] = results[2 * i]["out"] + results[2 * i + 1]["out"] + bo
    return out


def kernel(**inputs):
    kproj = pick_kproj(inputs["attention_mask"])
    nc = get_nc(kproj)
    in_maps = make_in_maps(kproj, **inputs)
    res = run_bass_kernel_spmd(nc, in_maps, core_ids=list(range(N_CORES)))
    return assemble(res.results, inputs["bo"])


# revision 14
# speedup vs baseline: 14101.8901x; 13881.6229x over previous
"""MemoryAttention Trainium2 Bass kernel.

Problem (hardcoded): b=4, nq=nk=2048, d_model=512, n_heads=8, d_k=64,
n_mem=64 memory slots appended to keys/values, per-key attention weights,
and a key mask (mask==1 -> -inf before softmax).

Sharding: 8 cores = (batch i in 0..3) x (head-group g in 0..1, 4 heads).
Host sums the two head-group partials per batch and adds bo.

Key trick: masked keys contribute exactly zero to the softmax, and the mask
is a kernel input - so the host COMPACTS keys per batch (keeps only the
~50% unmasked keys), cutting attention work ~2x. The kernel is compiled for
a fixed padded key count; a full-width variant is compiled as fallback if a
batch ever has too many unmasked keys.

Per-core pipeline (all matmuls float32r: 11-bit-mantissa RNE on ingest,
fp32 accumulate, 1 PE cycle/row):
  QT[dims,nq]   = Wq_g @ q^T + bq   (x^T supplied by host)
  KT[dims,kc]   = [Wk_g @ k_kept^T + bk | m_k*8 | 0]
  VC[keys,dims] = [v_kept @ Wv_g^T + bv ; m_v*8 ; 0] plus a ones column
  per (head, 1024-query tile, 128-key tile):
    S^T = KT_h.T @ QT_h                (PSUM, 2 matmuls of N=512)
    p~  = Exp(S^T * scale_k + bias_k)  (ONE 1024-wide ACT op; per-key
          attention weight & 1/sqrt(dk) in scale, mask/pad -1e30 in bias)
    pv += VC_h'.T @ p~                 (ones column accumulates denominator)
  nout = pv[0:64] * recip(denom)       (DVE; recip broadcast via GpSimd)
  out_partial += nout_pair.T @ WoT     (head pairs packed to K=128)

Self-contained: no file reads, shapes hardcoded.
"""

import numpy as np

import concourse.bass as bass
import concourse.tile as tile
import concourse.mybir as mybir
from concourse import bacc
from concourse.bass_utils import run_bass_kernel_spmd

F32 = mybir.dt.float32
F32R = mybir.dt.float32r
AF = mybir.ActivationFunctionType
ts = bass.ts

D_MODEL = 512
N_HEADS = 8
N_MEM = 64
DK = 64
B = 4
NQ = 2048
NK = 2048
NEG = -1.0e30

N_CORES = 8
HPG = 4              # heads per group
GD = HPG * DK        # 256 dims per group

KPROJ_COMPACT = 1152   # unmasked-key capacity (mean 1024, +5.7 sigma)


def build_nc(kproj):
    """kproj = projected-key columns (multiple of 128). Key layout:
    [0:kproj) compacted keys (+zero pad), [kproj:kproj+64) memory slots,
    [kproj+64:kproj+128) zero pad."""
    nkp = kproj + 128
    nkt = nkp // 128
    small = kproj > 1280   # wide fallback: shrink buffers to fit SBUF
    expp_bufs = 2 if small else 4
    recp_bufs = 1 if small else 2
    outp_bufs = 1 if small else 3
    noutp_bufs = 1 if small else 2
    sc_tag = "rec" if small else "sc"

    nc = bacc.Bacc("TRN2", target_bir_lowering=False, debug=False)

    qT = nc.dram_tensor("qT", [D_MODEL, NQ], F32R, kind="ExternalInput").ap()
    kT = nc.dram_tensor("kT", [D_MODEL, kproj], F32R, kind="ExternalInput").ap()
    vT = nc.dram_tensor("vT", [D_MODEL, kproj], F32R, kind="ExternalInput").ap()
    wqT = nc.dram_tensor("wqT", [D_MODEL, GD], F32R, kind="ExternalInput").ap()
    wkT = nc.dram_tensor("wkT", [D_MODEL, GD], F32R, kind="ExternalInput").ap()
    wvT = nc.dram_tensor("wvT", [D_MODEL, GD], F32R, kind="ExternalInput").ap()
    woT = nc.dram_tensor("woT", [GD, D_MODEL], F32R, kind="ExternalInput").ap()
    bq2 = nc.dram_tensor("bq2", [128, 2], F32, kind="ExternalInput").ap()
    bk2 = nc.dram_tensor("bk2", [128, 2], F32, kind="ExternalInput").ap()
    bvb = nc.dram_tensor("bvb", [GD], F32, kind="ExternalInput").ap()
    mkT = nc.dram_tensor("mkT", [2, 128, 128], F32R, kind="ExternalInput").ap()
    mv16 = nc.dram_tensor("mv16", [128, HPG, 65], F32R, kind="ExternalInput").ap()
    onesd = nc.dram_tensor("onesd", [1], F32R, kind="ExternalInput").ap()
    scalev = nc.dram_tensor("scalev", [nkp], F32, kind="ExternalInput").ap()
    biasv = nc.dram_tensor("biasv", [nkp], F32, kind="ExternalInput").ap()
    out = nc.dram_tensor("out", [NQ, D_MODEL], F32, kind="ExternalOutput").ap()

    with tile.TileContext(nc) as tc:
        with tc.tile_pool(name="const", bufs=1) as const, \
             tc.tile_pool(name="stage", bufs=1) as stage, \
             tc.tile_pool(name="expp", bufs=expp_bufs) as expp, \
             tc.tile_pool(name="noutp", bufs=noutp_bufs) as noutp, \
             tc.tile_pool(name="recp", bufs=recp_bufs) as recp, \
             tc.tile_pool(name="outp", bufs=outp_bufs) as outp, \
             tc.tile_pool(name="ps_st", bufs=2, space="PSUM") as ps_st, \
             tc.tile_pool(name="ps_pv", bufs=2, space="PSUM") as ps_pv:

            # ---- weights / constants ----
            wq_sb = const.tile([128, 4, GD], F32R, tag="wq")
            wk_sb = const.tile([128, 4, GD], F32R, tag="wk")
            wv_sb = const.tile([128, 4, GD], F32R, tag="wv")
            wo_sb = const.tile([128, 2, D_MODEL], F32R, tag="wo")
            nc.sync.dma_start(wq_sb[:], wqT.rearrange("(ic p) m -> p ic m", p=128))
            nc.sync.dma_start(wk_sb[:], wkT.rearrange("(ic p) m -> p ic m", p=128))
            nc.sync.dma_start(wv_sb[:], wvT.rearrange("(ic p) m -> p ic m", p=128))
            nc.sync.dma_start(wo_sb[:], woT.rearrange("(c p) n -> p c n", p=128))
            bq_sb = const.tile([128, 2], F32, tag="bq")
            bk_sb = const.tile([128, 2], F32, tag="bk")
            nc.sync.dma_start(bq_sb[:], bq2)
            nc.sync.dma_start(bk_sb[:], bk2)
            bvb_sb = const.tile([128, HPG, DK], F32, tag="bvb")
            nc.sync.dma_start(
                bvb_sb[:],
                bvb.rearrange("(h d) -> h d", h=HPG).unsqueeze(0)
                   .broadcast_to([128, HPG, DK]))
            scale_sb = const.tile([128, nkt], F32, tag="scale")
            bias_sb = const.tile([128, nkt], F32, tag="bias")
            nc.sync.dma_start(scale_sb[:], scalev.rearrange("(t p) -> p t", p=128))
            nc.sync.dma_start(bias_sb[:], biasv.rearrange("(t p) -> p t", p=128))

            # ---- persistent projected tensors ----
            QT = [const.tile([128, NQ], F32R, tag=f"QT{c}", name=f"QT{c}")
                  for c in range(2)]
            KT = [const.tile([128, nkp], F32R, tag=f"KT{c}", name=f"KT{c}")
                  for c in range(2)]
            VC = const.tile([128, nkt, HPG, 65], F32R, tag="VC")

            for c in range(2):
                nc.sync.dma_start(KT[c][:, kproj:nkp], mkT[c])
            for h in range(HPG):
                nc.sync.dma_start(
                    VC[:, 0:nkt - 1, h, 64:65],
                    onesd.unsqueeze(0).unsqueeze(0)
                         .broadcast_to([128, nkt - 1, 1]))
            nc.sync.dma_start(VC[:, nkt - 1, :, :], mv16)

            # ---- stage q/k/v (host-transposed) ----
            qw = NQ // 2 if small else NQ
            qs = [stage.tile([128, qw], F32R, tag=f"qs{ic}", name=f"qs{ic}")
                  for ic in range(4)]
            ks = [stage.tile([128, kproj], F32R, tag=f"ks{ic}", name=f"ks{ic}")
                  for ic in range(4)]
            vs = [stage.tile([128, kproj], F32R, tag=f"vs{ic}", name=f"vs{ic}")
                  for ic in range(4)]
            for ic in range(4):
                nc.sync.dma_start(ks[ic][:], kT[ts(ic, 128), :])
            for ic in range(4):
                nc.sync.dma_start(vs[ic][:], vT[ts(ic, 128), :])
            if not small:
                for ic in range(4):
                    nc.sync.dma_start(qs[ic][:], qT[ts(ic, 128), :])

            # ---- projections ----
            # KT/VC first: they gate the whole attention phase; QT last
            # (q is the largest input DMA and only gates its own J-tile).
            for c in range(2):
                j0 = 0
                while j0 < kproj:
                    jb = min(512, kproj - j0)
                    ps = ps_pv.tile([128, jb], F32, tag="pv", name="psk")
                    for ic in range(4):
                        nc.tensor.matmul(ps[:], wk_sb[:, ic, ts(c, 128)],
                                         ks[ic][:, j0:j0 + jb],
                                         start=(ic == 0), stop=(ic == 3))
                    nc.vector.tensor_scalar_add(KT[c][:, j0:j0 + jb], ps[:],
                                                bk_sb[:, c:c + 1])
                    j0 += jb
            for kt in range(kproj // 128):
                ps = ps_pv.tile([128, GD], F32, tag="pv", name="psv")
                for ic in range(4):
                    nc.tensor.matmul(ps[:], vs[ic][:, ts(kt, 128)],
                                     wv_sb[:, ic, :],
                                     start=(ic == 0), stop=(ic == 3))
                nc.vector.tensor_add(VC[:, kt, :, 0:64],
                                     ps[:].rearrange("p (h d) -> p h d", h=HPG),
                                     bvb_sb[:])
            for qh in range(NQ // qw):
                if small:
                    for ic in range(4):
                        nc.sync.dma_start(qs[ic][:],
                                          qT[ts(ic, 128), ts(qh, qw)])
                for c in range(2):
                    for j in range(qw // 512):
                        ps = ps_pv.tile([128, 512], F32, tag="pv", name="psq")
                        for ic in range(4):
                            nc.tensor.matmul(ps[:], wq_sb[:, ic, ts(c, 128)],
                                             qs[ic][:, ts(j, 512)],
                                             start=(ic == 0), stop=(ic == 3))
                        nc.vector.tensor_scalar_add(
                            QT[c][:, qh * qw + j * 512:qh * qw + (j + 1) * 512],
                            ps[:], bq_sb[:, c:c + 1])
            # ---- attention ----
            for J in range(NQ // 1024):
                q0 = J * 1024
                nops = [noutp.tile([128, 1024], F32R, tag=f"nop{c}",
                                   name=f"nop{c}") for c in range(2)]
                for h in (1, 3, 0, 2):
                    c, r = divmod(h, 2)
                    base = 64 * r
                    pv = ps_pv.tile([65, 1024], F32, tag="pv", name="pv")
                    for kt in range(nkt):
                        st = ps_st.tile([128, 1024], F32, tag="st", name="st")
                        for u in range(2):
                            nc.tensor.matmul(
                                st[:, ts(u, 512)],
                                KT[c][base:base + 64, ts(kt, 128)],
                                QT[c][base:base + 64,
                                      q0 + u * 512:q0 + (u + 1) * 512],
                                start=True, stop=True)
                        ex = expp.tile([128, 1024], F32R, tag="ex", name="ex")
                        nc.scalar.activation(
                            ex[:], st[:], AF.Exp,
                            bias=bias_sb[:, kt:kt + 1],
                            scale=scale_sb[:, kt:kt + 1])
                        for u in range(2):
                            nc.tensor.matmul(pv[:, ts(u, 512)],
                                             VC[:, kt, h, :], ex[:, ts(u, 512)],
                                             start=(kt == 0),
                                             stop=(kt == nkt - 1))
                    rec = recp.tile([65, 1024], F32, tag="rec", name="rec")
                    nc.vector.reciprocal(rec[64:65, :], pv[64:65, :])
                    # partition_broadcast ucode only reads partition 0 on HW:
                    # hop the reciprocal row down to partition 0 via DMA first
                    rec0 = recp.tile([1, 1024], F32, tag="rec0", name="rec0")
                    nc.gpsimd.dma_start(rec0[:], rec[64:65, :])
                    rb = recp.tile([64, 1024], F32, tag="rb", name="rb")
                    nc.gpsimd.partition_broadcast(rb[:], rec0[0:1, :])
                    if r == 0:
                        nc.vector.tensor_mul(nops[c][0:64, :], pv[0:64, :],
                                             rb[:])
                    else:
                        sc = recp.tile([64, 1024], F32R, tag=sc_tag, name="sc")
                        nc.vector.tensor_mul(sc[:], pv[0:64, :], rb[:])
                        nc.gpsimd.dma_start(nops[c][64:128, :], sc[:])
                for qc in range(8):
                    pf = ps_pv.tile([128, 512], F32, tag="pv", name="pf")
                    for c in range(2):
                        nc.tensor.matmul(pf[:], nops[c][:, ts(qc, 128)],
                                         wo_sb[:, c, :],
                                         start=(c == 0), stop=(c == 1))
                    ob = outp.tile([128, 512], F32, tag="ob", name="ob")
                    nc.scalar.copy(ob[:], pf[:])
                    nc.sync.dma_start(out[q0 + qc * 128:q0 + (qc + 1) * 128, :],
                                      ob[:])

    nc.compile()
    return nc


_NCS = {}


def get_nc(kproj=KPROJ_COMPACT):
    if kproj not in _NCS:
        _NCS[kproj] = build_nc(kproj)
    return _NCS[kproj]


def make_in_maps(kproj, q, k, v, attention_mask, attention_weights,
                 Wq, bq, Wk, bk, Wv, bv, Wo, bo, m_k, m_v):
    q = np.asarray(q, np.float32)
    k = np.asarray(k, np.float32)
    v = np.asarray(v, np.float32)
    Wq, Wk, Wv, Wo = (np.asarray(x, np.float32) for x in (Wq, Wk, Wv, Wo))
    m_k = np.asarray(m_k, np.float32)
    m_v = np.asarray(m_v, np.float32)
    aw = np.asarray(attention_weights, np.float32).reshape(B, NK)
    am = np.asarray(attention_mask).reshape(B, NK)
    nkp = kproj + 128
    compact = kproj < NK

    mks = m_k[0] * np.float32(np.sqrt(DK))      # [64, 512]
    mvs = m_v[0] * np.float32(np.sqrt(N_MEM))   # [64, 512]

    in_maps = []
    for i in range(B):
        if compact:
            idx = np.flatnonzero(am[i] == 0)
        else:
            idx = np.arange(NK)
        nkeep = len(idx)
        assert nkeep <= kproj, (nkeep, kproj)
        kTc = np.zeros((D_MODEL, kproj), np.float32)
        vTc = np.zeros((D_MODEL, kproj), np.float32)
        kTc[:, :nkeep] = k[i].T[:, idx]
        vTc[:, :nkeep] = v[i].T[:, idx]
        qTc = np.ascontiguousarray(q[i].T)

        scalev = np.zeros(nkp, np.float32)
        biasv = np.full(nkp, np.float32(NEG), np.float32)
        scalev[:nkeep] = aw[i, idx] * np.float32(1.0 / 8.0)
        biasv[:nkeep] = np.where(am[i, idx] != 0, np.float32(NEG), 0.0)
        scalev[kproj:kproj + N_MEM] = np.float32(1.0 / 8.0)
        biasv[kproj:kproj + N_MEM] = 0.0

        for g in range(2):
            sl = slice(g * GD, (g + 1) * GD)
            mkT_full = np.zeros((2, 128, 128), np.float32)
            mkT_full[:, :, :N_MEM] = mks[:, sl].T.reshape(2, 128, N_MEM)
            mv16_full = np.zeros((128, HPG, 65), np.float32)
            mv16_full[:N_MEM, :, :DK] = mvs[:, sl].reshape(N_MEM, HPG, DK)
            mv16_full[:N_MEM, :, DK] = 1.0
            in_maps.append(dict(
                qT=qTc, kT=kTc, vT=vTc,
                wqT=np.ascontiguousarray(Wq[sl, :].T),
                wkT=np.ascontiguousarray(Wk[sl, :].T),
                wvT=np.ascontiguousarray(Wv[sl, :].T),
                woT=np.ascontiguousarray(Wo[:, sl].T),
                bq2=np.ascontiguousarray(
                    np.asarray(bq, np.float32)[sl].reshape(2, 128).T),
                bk2=np.ascontiguousarray(
                    np.asarray(bk, np.float32)[sl].reshape(2, 128).T),
                bvb=np.ascontiguousarray(np.asarray(bv, np.float32)[sl]),
                mkT=mkT_full,
                mv16=mv16_full,
                onesd=np.ones(1, np.float32),
                scalev=scalev, biasv=biasv,
            ))
    return in_maps


def pick_kproj(attention_mask):
    am = np.asarray(attention_mask).reshape(B, NK)
    max_keep = max(int((am[i] == 0).sum()) for i in range(B))
    for tier in (KPROJ_COMPACT, 1280):
        if max_keep <= tier:
            return tier
    return NK


def assemble(results, bo):
    bo = np.asarray(bo, np.float32)
    out = np.empty((B, NQ, D_MODEL), np.float32)
    for i in range(B):
        out[i] = results[2 * i]["out"] + results[2 * i + 1]["out"] + bo
    return out


def kernel(**inputs):
    kproj = pick_kproj(inputs["attention_mask"])
    nc = get_nc(kproj)
    in_maps = make_in_maps(kproj, **inputs)
    res = run_bass_kernel_spmd(nc, in_maps, core_ids=list(range(N_CORES)))
    return assemble(res.results, inputs["bo"])


# revision 17
# speedup vs baseline: 14547.6710x; 1.0316x over previous
"""MemoryAttention Trainium2 Bass kernel.

Problem (hardcoded): b=4, nq=nk=2048, d_model=512, n_heads=8, d_k=64,
n_mem=64 memory slots appended to keys/values, per-key attention weights,
and a key mask (mask==1 -> -inf before softmax).

Sharding: 8 cores = (batch i in 0..3) x (head-group g in 0..1, 4 heads).
Host sums the two head-group partials per batch and adds bo.

Key trick: masked keys contribute exactly zero to the softmax, and the mask
is a kernel input - so the host COMPACTS keys per batch (keeps only the
~50% unmasked keys), cutting attention work ~2x. The kernel is compiled for
a fixed padded key count; a full-width variant is compiled as fallback if a
batch ever has too many unmasked keys.

Per-core pipeline (all matmuls float32r: 11-bit-mantissa RNE on ingest,
fp32 accumulate, 1 PE cycle/row):
  QT[dims,nq]   = Wq_g @ q^T + bq   (x^T supplied by host)
  KT[dims,kc]   = [Wk_g @ k_kept^T + bk | m_k*8 | 0]
  VC[keys,dims] = [v_kept @ Wv_g^T + bv ; m_v*8 ; 0] plus a ones column
  per (head, 1024-query tile, 128-key tile):
    S^T = KT_h.T @ QT_h                (PSUM, 2 matmuls of N=512)
    p~  = Exp(S^T * scale_k + bias_k)  (ONE 1024-wide ACT op; per-key
          attention weight & 1/sqrt(dk) in scale, mask/pad -1e30 in bias)
    pv += VC_h'.T @ p~                 (ones column accumulates denominator)
  nout = pv[0:64] * recip(denom)       (DVE; recip broadcast via GpSimd)
  out_partial += nout_pair.T @ WoT     (head pairs packed to K=128)

Self-contained: no file reads, shapes hardcoded.
"""

import numpy as np

import concourse.bass as bass
import concourse.tile as tile
import concourse.mybir as mybir
from concourse import bacc
from concourse.bass_utils import run_bass_kernel_spmd

F32 = mybir.dt.float32
F32R = mybir.dt.float32r
AF = mybir.ActivationFunctionType
ts = bass.ts

D_MODEL = 512
N_HEADS = 8
N_MEM = 64
DK = 64
B = 4
NQ = 2048
NK = 2048
NEG = -1.0e30

N_CORES = 8
HPG = 4              # heads per group
GD = HPG * DK        # 256 dims per group

KPROJ_COMPACT = 1152   # unmasked-key capacity (mean 1024, +5.7 sigma)


def build_nc(kproj):
    """kproj = projected-key columns (multiple of 128). Key layout:
    [0:kproj) compacted keys (+zero pad), [kproj:kproj+64) memory slots,
    [kproj+64:kproj+128) zero pad."""
    nkp = kproj + 128
    nkt = nkp // 128
    small = kproj > 1280   # wide fallback: shrink buffers to fit SBUF
    expp_bufs = 2 if small else 4
    recp_bufs = 1 if small else 2
    outp_bufs = 1 if small else 3
    noutp_bufs = 1 if small else 2
    sc_tag = "rec" if small else "sc"

    nc = bacc.Bacc("TRN2", target_bir_lowering=False, debug=False)

    qT = nc.dram_tensor("qT", [D_MODEL, NQ], F32R, kind="ExternalInput").ap()
    kT = nc.dram_tensor("kT", [D_MODEL, kproj], F32R, kind="ExternalInput").ap()
    vT = nc.dram_tensor("vT", [D_MODEL, kproj], F32R, kind="ExternalInput").ap()
    wqT = nc.dram_tensor("wqT", [D_MODEL, GD], F32R, kind="ExternalInput").ap()
    wkT = nc.dram_tensor("wkT", [D_MODEL, GD], F32R, kind="ExternalInput").ap()
    wvT = nc.dram_tensor("wvT", [D_MODEL, GD], F32R, kind="ExternalInput").ap()
    woT = nc.dram_tensor("woT", [GD, D_MODEL], F32R, kind="ExternalInput").ap()
    bq2 = nc.dram_tensor("bq2", [128, 2], F32, kind="ExternalInput").ap()
    bk2 = nc.dram_tensor("bk2", [128, 2], F32, kind="ExternalInput").ap()
    bvb = nc.dram_tensor("bvb", [GD], F32, kind="ExternalInput").ap()
    mkT = nc.dram_tensor("mkT", [2, 128, 128], F32R, kind="ExternalInput").ap()
    mv16 = nc.dram_tensor("mv16", [128, HPG, 65], F32R, kind="ExternalInput").ap()
    onesd = nc.dram_tensor("onesd", [1], F32R, kind="ExternalInput").ap()
    scalev = nc.dram_tensor("scalev", [nkp], F32, kind="ExternalInput").ap()
    biasv = nc.dram_tensor("biasv", [nkp], F32, kind="ExternalInput").ap()
    out = nc.dram_tensor("out", [NQ, D_MODEL], F32, kind="ExternalOutput").ap()

    with tile.TileContext(nc) as tc:
        with tc.tile_pool(name="const", bufs=1) as const, \
             tc.tile_pool(name="stage", bufs=1) as stage, \
             tc.tile_pool(name="expp", bufs=expp_bufs) as expp, \
             tc.tile_pool(name="noutp", bufs=noutp_bufs) as noutp, \
             tc.tile_pool(name="recp", bufs=recp_bufs) as recp, \
             tc.tile_pool(name="outp", bufs=outp_bufs) as outp, \
             tc.tile_pool(name="ps_st", bufs=2, space="PSUM") as ps_st, \
             tc.tile_pool(name="ps_pv", bufs=2, space="PSUM") as ps_pv:

            # ---- weights / constants ----
            wq_sb = const.tile([128, 4, GD], F32R, tag="wq")
            wk_sb = const.tile([128, 4, GD], F32R, tag="wk")
            wv_sb = const.tile([128, 4, GD], F32R, tag="wv")
            wo_sb = const.tile([128, 2, D_MODEL], F32R, tag="wo")
            nc.sync.dma_start(wq_sb[:], wqT.rearrange("(ic p) m -> p ic m", p=128))
            nc.sync.dma_start(wk_sb[:], wkT.rearrange("(ic p) m -> p ic m", p=128))
            nc.sync.dma_start(wv_sb[:], wvT.rearrange("(ic p) m -> p ic m", p=128))
            nc.sync.dma_start(wo_sb[:], woT.rearrange("(c p) n -> p c n", p=128))
            bq_sb = const.tile([128, 2], F32, tag="bq")
            bk_sb = const.tile([128, 2], F32, tag="bk")
            nc.sync.dma_start(bq_sb[:], bq2)
            nc.sync.dma_start(bk_sb[:], bk2)
            bvb_sb = const.tile([128, HPG, DK], F32, tag="bvb")
            nc.sync.dma_start(
                bvb_sb[:],
                bvb.rearrange("(h d) -> h d", h=HPG).unsqueeze(0)
                   .broadcast_to([128, HPG, DK]))
            scale_sb = const.tile([128, nkt], F32, tag="scale")
            bias_sb = const.tile([128, nkt], F32, tag="bias")
            nc.sync.dma_start(scale_sb[:], scalev.rearrange("(t p) -> p t", p=128))
            nc.sync.dma_start(bias_sb[:], biasv.rearrange("(t p) -> p t", p=128))

            # ---- persistent projected tensors ----
            QT = [const.tile([128, NQ], F32R, tag=f"QT{c}", name=f"QT{c}")
                  for c in range(2)]
            KT = [const.tile([128, nkp], F32R, tag=f"KT{c}", name=f"KT{c}")
                  for c in range(2)]
            VC = const.tile([128, nkt, HPG, 65], F32R, tag="VC")

            for c in range(2):
                nc.sync.dma_start(KT[c][:, kproj:nkp], mkT[c])
            for h in range(HPG):
                nc.sync.dma_start(
                    VC[:, 0:nkt - 1, h, 64:65],
                    onesd.unsqueeze(0).unsqueeze(0)
                         .broadcast_to([128, nkt - 1, 1]))
            nc.sync.dma_start(VC[:, nkt - 1, :, :], mv16)

            # ---- stage q/k/v (host-transposed) ----
            qw = NQ // 2 if small else NQ
            qs = [stage.tile([128, qw], F32R, tag=f"qs{ic}", name=f"qs{ic}")
                  for ic in range(4)]
            ks = [stage.tile([128, kproj], F32R, tag=f"ks{ic}", name=f"ks{ic}")
                  for ic in range(4)]
            vs = [stage.tile([128, kproj], F32R, tag=f"vs{ic}", name=f"vs{ic}")
                  for ic in range(4)]
            kh = kproj // 2
            for u in range(2):
                for ic in range(4):
                    nc.sync.dma_start(ks[ic][:, ts(u, kh)],
                                      kT[ts(ic, 128), ts(u, kh)])
            for u in range(2):
                for ic in range(4):
                    nc.sync.dma_start(vs[ic][:, ts(u, kh)],
                                      vT[ts(ic, 128), ts(u, kh)])
            if not small:
                for u in range(2):
                    for ic in range(4):
                        nc.sync.dma_start(qs[ic][:, ts(u, NQ // 2)],
                                          qT[ts(ic, 128), ts(u, NQ // 2)])

            # ---- projections ----
            # KT/VC first: they gate the whole attention phase; QT last
            # (q is the largest input DMA and only gates its own J-tile).
            for c in range(2):
                j0 = 0
                while j0 < kproj:
                    jb = min(512, kproj - j0)
                    ps = ps_pv.tile([128, jb], F32, tag="pv", name="psk")
                    for ic in range(4):
                        nc.tensor.matmul(ps[:], wk_sb[:, ic, ts(c, 128)],
                                         ks[ic][:, j0:j0 + jb],
                                         start=(ic == 0), stop=(ic == 3))
                    nc.vector.tensor_scalar_add(KT[c][:, j0:j0 + jb], ps[:],
                                                bk_sb[:, c:c + 1])
                    j0 += jb
            for kt in range(kproj // 128):
                ps = ps_pv.tile([128, GD], F32, tag="pv", name="psv")
                for ic in range(4):
                    nc.tensor.matmul(ps[:], vs[ic][:, ts(kt, 128)],
                                     wv_sb[:, ic, :],
                                     start=(ic == 0), stop=(ic == 3))
                nc.vector.tensor_add(VC[:, kt, :, 0:64],
                                     ps[:].rearrange("p (h d) -> p h d", h=HPG),
                                     bvb_sb[:])
            for qh in range(NQ // qw):
                if small:
                    for ic in range(4):
                        nc.sync.dma_start(qs[ic][:],
                                          qT[ts(ic, 128), ts(qh, qw)])
                for c in range(2):
                    for j in range(qw // 512):
                        ps = ps_pv.tile([128, 512], F32, tag="pv", name="psq")
                        for ic in range(4):
                            nc.tensor.matmul(ps[:], wq_sb[:, ic, ts(c, 128)],
                                             qs[ic][:, ts(j, 512)],
                                             start=(ic == 0), stop=(ic == 3))
                        nc.vector.tensor_scalar_add(
                            QT[c][:, qh * qw + j * 512:qh * qw + (j + 1) * 512],
                            ps[:], bq_sb[:, c:c + 1])
            # ---- attention ----
            for J in range(NQ // 1024):
                q0 = J * 1024
                nops = [noutp.tile([128, 1024], F32R, tag=f"nop{c}",
                                   name=f"nop{c}") for c in range(2)]
                for h in (1, 3, 0, 2):
                    c, r = divmod(h, 2)
                    base = 64 * r
                    pv = ps_pv.tile([65, 1024], F32, tag="pv", name="pv")
                    for kt in range(nkt):
                        st = ps_st.tile([128, 1024], F32, tag="st", name="st")
                        for u in range(2):
                            nc.tensor.matmul(
                                st[:, ts(u, 512)],
                                KT[c][base:base + 64, ts(kt, 128)],
                                QT[c][base:base + 64,
                                      q0 + u * 512:q0 + (u + 1) * 512],
                                start=True, stop=True)
                        ex = expp.tile([128, 1024], F32R, tag="ex", name="ex")
                        nc.scalar.activation(
                            ex[:], st[:], AF.Exp,
                            bias=bias_sb[:, kt:kt + 1],
                            scale=scale_sb[:, kt:kt + 1])
                        for u in range(2):
                            nc.tensor.matmul(pv[:, ts(u, 512)],
                                             VC[:, kt, h, :], ex[:, ts(u, 512)],
                                             start=(kt == 0),
                                             stop=(kt == nkt - 1))
                    rec = recp.tile([65, 1024], F32, tag="rec", name="rec")
                    nc.vector.reciprocal(rec[64:65, :], pv[64:65, :])
                    # partition_broadcast ucode only reads partition 0 on HW:
                    # hop the reciprocal row down to partition 0 via DMA first
                    rec0 = recp.tile([1, 1024], F32, tag="rec0", name="rec0")
                    nc.gpsimd.dma_start(rec0[:], rec[64:65, :])
                    rb = recp.tile([64, 1024], F32, tag="rb", name="rb")
                    nc.gpsimd.partition_broadcast(rb[:], rec0[0:1, :])
                    if r == 0:
                        nc.vector.tensor_mul(nops[c][0:64, :], pv[0:64, :],
                                             rb[:])
                    else:
                        sc = recp.tile([64, 1024], F32R, tag=sc_tag, name="sc")
                        nc.vector.tensor_mul(sc[:], pv[0:64, :], rb[:])
                        nc.gpsimd.dma_start(nops[c][64:128, :], sc[:])
                for qc in range(8):
                    pf = ps_pv.tile([128, 512], F32, tag="pv", name="pf")
                    for c in range(2):
                        nc.tensor.matmul(pf[:], nops[c][:, ts(qc, 128)],
                                         wo_sb[:, c, :],
                                         start=(c == 0), stop=(c == 1))
                    ob = outp.tile([128, 512], F32, tag="ob", name="ob")
                    nc.scalar.copy(ob[:], pf[:])
                    nc.sync.dma_start(out[q0 + qc * 128:q0 + (qc + 1) * 128, :],
                                      ob[:])

    nc.compile()
    return nc


_NCS = {}


def get_nc(kproj=KPROJ_COMPACT):
    if kproj not in _NCS:
        _NCS[kproj] = build_nc(kproj)
    return _NCS[kproj]


def make_in_maps(kproj, q, k, v, attention_mask, attention_weights,
                 Wq, bq, Wk, bk, Wv, bv, Wo, bo, m_k, m_v):
    q = np.asarray(q, np.float32)
    k = np.asarray(k, np.float32)
    v = np.asarray(v, np.float32)
    Wq, Wk, Wv, Wo = (np.asarray(x, np.float32) for x in (Wq, Wk, Wv, Wo))
    m_k = np.asarray(m_k, np.float32)
    m_v = np.asarray(m_v, np.float32)
    aw = np.asarray(attention_weights, np.float32).reshape(B, NK)
    am = np.asarray(attention_mask).reshape(B, NK)
    nkp = kproj + 128
    compact = kproj < NK

    mks = m_k[0] * np.float32(np.sqrt(DK))      # [64, 512]
    mvs = m_v[0] * np.float32(np.sqrt(N_MEM))   # [64, 512]

    in_maps = []
    for i in range(B):
        if compact:
            idx = np.flatnonzero(am[i] == 0)
        else:
            idx = np.arange(NK)
        nkeep = len(idx)
        assert nkeep <= kproj, (nkeep, kproj)
        kTc = np.zeros((D_MODEL, kproj), np.float32)
        vTc = np.zeros((D_MODEL, kproj), np.float32)
        kTc[:, :nkeep] = k[i].T[:, idx]
        vTc[:, :nkeep] = v[i].T[:, idx]
        qTc = np.ascontiguousarray(q[i].T)

        scalev = np.zeros(nkp, np.float32)
        biasv = np.full(nkp, np.float32(NEG), np.float32)
        scalev[:nkeep] = aw[i, idx] * np.float32(1.0 / 8.0)
        biasv[:nkeep] = np.where(am[i, idx] != 0, np.float32(NEG), 0.0)
        scalev[kproj:kproj + N_MEM] = np.float32(1.0 / 8.0)
        biasv[kproj:kproj + N_MEM] = 0.0

        for g in range(2):
            sl = slice(g * GD, (g + 1) * GD)
            mkT_full = np.zeros((2, 128, 128), np.float32)
            mkT_full[:, :, :N_MEM] = mks[:, sl].T.reshape(2, 128, N_MEM)
            mv16_full = np.zeros((128, HPG, 65), np.float32)
            mv16_full[:N_MEM, :, :DK] = mvs[:, sl].reshape(N_MEM, HPG, DK)
            mv16_full[:N_MEM, :, DK] = 1.0
            in_maps.append(dict(
                qT=qTc, kT=kTc, vT=vTc,
                wqT=np.ascontiguousarray(Wq[sl, :].T),
                wkT=np.ascontiguousarray(Wk[sl, :].T),
                wvT=np.ascontiguousarray(Wv[sl, :].T),
                woT=np.ascontiguousarray(Wo[:, sl].T),
                bq2=np.ascontiguousarray(
                    np.asarray(bq, np.float32)[sl].reshape(2, 128).T),
                bk2=np.ascontiguousarray(
                    np.asarray(bk, np.float32)[sl].reshape(2, 128).T),
                bvb=np.ascontiguousarray(np.asarray(bv, np.float32)[sl]),
                mkT=mkT_full,
                mv16=mv16_full,
                onesd=np.ones(1, np.float32),
                scalev=scalev, biasv=biasv,
            ))
    return in_maps


def pick_kproj(attention_mask):
    am = np.asarray(attention_mask).reshape(B, NK)
    max_keep = max(int((am[i] == 0).sum()) for i in range(B))
    for tier in (KPROJ_COMPACT, 1280):
        if max_keep <= tier:
            return tier
    return NK


def assemble(results, bo):
    bo = np.asarray(bo, np.float32)
    out = np.empty((B, NQ, D_MODEL), np.float32)
    for i in range(B):
        out[i] = results[2 * i]["out"] + results[2 * i + 1]["out"] + bo
    return out


def kernel(**inputs):
    kproj = pick_kproj(inputs["attention_mask"])
    nc = get_nc(kproj)
    in_maps = make_in_maps(kproj, **inputs)
    res = run_bass_kernel_spmd(nc, in_maps, core_ids=list(range(N_CORES)))
    return assemble(res.results, inputs["bo"])
